# revision 4
# baseline (speedup 1.0000x reference)
"""Multi-head attention (B=2, S=2048, H=8, Dh=32, D=256) on 8 TRN2 NeuronCores.

Sharding: core c -> (batch b = c//4, query-block qb = c%4 of 512 rows).
Each core computes full attention + output projection for its 512 query rows;
no cross-core communication is needed.  Host does layout prep only
(transposes + bf16 casts); all FLOPs run on device.

Device-side layout (per core):
  - raw activations shipped transposed: qT [256f, 512q], kT/vT [256f, 2048s]
  - q/k projections produced directly transposed (head-dim on partitions,
    head h at partitions 32*(h%4) of free-block h//4) so QK^T runs as
    K=32 row-tiled matmuls; the two heads of a pair are issued
    back-to-back at row positions 32r so the PE runs them concurrently.
  - scores computed TRANSPOSED: scoresT[k, q]; exp is a straight ScalarE
    pass over PSUM (no max subtraction: scores ~ N(0,1)).
  - v projected to natural layout augmented with a ones column per head
    ([128, 16, 8, 33]) so each PV matmul (M=33) also accumulates the
    softmax denominator as its last output row - no separate reduction.
  - normalization: reciprocal of the two denominator rows (partitions 32
    and 96) + K=1 matmul row-broadcast + one DVE multiply per band.
  - final projection: K=32 matmuls per head slice against a host-permuted
    WoT whose row bands match the PV output partition bases.

Schedule (v2): ScalarE is the target critical path (~8.4M exps/core at
1 elem/lane/cycle + ~170cyc/instr overhead -> ~64us).  Its stream carries
ONLY the exp ACTIVATEs (DMA triggers live on sync/vector/gpsimd queues).
Exp regions are [128, 2(heads), 512] PSUM slots (one k-chunk x head-pair,
N=1024); QK production runs 2 slots ahead.  Projections (phase 1-2) and
PV/normalize (phases 2-4) are spread between QK groups so each phase's PE
load roughly matches the ScalarE drain rate, keeping the PE stream dense
(HAM stays released) without starving exp.  The output projection runs in
the tail on freed PSUM slots.  PSUM budget: 3x[128,2,512] rotating slots
(scores + transients + broadcast + final o=0) and 2x[128,512] accumulator
slots (PV quads in flight + final o=1) = exactly 8 banks.
"""

import sys

sys.path.insert(0, "/opt/trn_rl_repo")

import numpy as np
import ml_dtypes

import concourse.bass as bass
import concourse.bacc as bacc
import concourse.mybir as mybir
from concourse.tile import TileContext
from concourse.bass import ts
from concourse.bass_utils import run_bass_kernel_spmd

BF16 = mybir.dt.bfloat16
F32 = mybir.dt.float32
EXP = mybir.ActivationFunctionType.Exp

B, SEQ, D = 2, 2048, 256
H, DH = 8, 32
QB = 512  # query rows per core
NKT = SEQ // 128  # 16 k-chunk tiles (partition tiles of scoresT)


def _build_graph():
    nc = bacc.Bacc("TRN2", target_bir_lowering=False, debug=False)

    qT = nc.declare_dram_parameter("qT", [D, QB], BF16, isOutput=False)
    kT = nc.declare_dram_parameter("kT", [D, SEQ], BF16, isOutput=False)
    vT = nc.declare_dram_parameter("vT", [D, SEQ], BF16, isOutput=False)
    wqT = nc.declare_dram_parameter("wqT", [D, D], BF16, isOutput=False)
    wkT = nc.declare_dram_parameter("wkT", [D, D], BF16, isOutput=False)
    wvT = nc.declare_dram_parameter("wvT", [D + 1, H * (DH + 1)], BF16, isOutput=False)
    woP = nc.declare_dram_parameter("woP", [128, 4 * D], BF16, isOutput=False)
    bo = nc.declare_dram_parameter("bo", [D, 1], F32, isOutput=False)
    outT = nc.declare_dram_parameter("outT", [D, QB], F32, isOutput=True)

    with TileContext(nc) as tc:
        with (
            tc.tile_pool(name="cst", bufs=1) as cst,
            tc.tile_pool(name="sb", bufs=1) as sb,
            tc.tile_pool(name="ps", bufs=2, space="PSUM") as ps,
        ):
            # warm loads the exp table set early so the ~2.7us
            # ACT_TABLE_LOAD overlaps the DMA/projection phase.
            warm = cst.tile([1, 1], F32)
            nc.vector.memset(warm[:], 0.0)
            nc.scalar.activation(warm[:], warm[:], EXP)

            # ---- inputs.  k-path on the sync HWDGE queue, q-path on the
            # vector queue, v-path + weights on the gpsimd (SWDGE) queue.
            # NOTHING on the scalar queue - it must only carry exps.
            wk_t = cst.tile([128, 2, D], BF16)
            wq_t = cst.tile([128, 2, D], BF16)
            kT_t = cst.tile([128, 2, SEQ], BF16)
            qT_t = cst.tile([128, 2, QB], BF16)
            wv_t = cst.tile([128, 2, H * (DH + 1)], BF16)
            wva_t = cst.tile([1, H * (DH + 1)], BF16)
            vT_t = cst.tile([128, 2, SEQ], BF16)
            wo_t = cst.tile([128, 4, D], BF16)
            bo_t = cst.tile([128, 2, 1], F32)

            for f in range(2):
                nc.sync.dma_start(wk_t[:, f, :], wkT[ts(f, 128), :])
            for f in range(2):
                nc.gpsimd.dma_start(wq_t[:, f, :], wqT[ts(f, 128), :])
                nc.gpsimd.dma_start(qT_t[:, f, :], qT[ts(f, 128), :])
            for s4 in range(4):
                for f in range(2):
                    nc.sync.dma_start(
                        kT_t[:, f, ts(s4, 512)], kT[ts(f, 128), ts(s4, 512)]
                    )
            for f in range(2):
                nc.gpsimd.dma_start(wv_t[:, f, :], wvT[ts(f, 128), :])
            nc.gpsimd.dma_start(wva_t[:], wvT[D : D + 1, :])
            for s4 in range(4):
                for f in range(2):
                    nc.gpsimd.dma_start(
                        vT_t[:, f, ts(s4, 512)], vT[ts(f, 128), ts(s4, 512)]
                    )
            nc.gpsimd.dma_start(wo_t[:, :, :], woP.rearrange("p (b d) -> p b d", b=4))
            for o in range(2):
                nc.gpsimd.dma_start(bo_t[:, o, :], bo[ts(o, 128), :])

            # ---- SBUF destinations for projections.
            qp = cst.tile([128, 2, QB], BF16)  # q_projT
            kp = cst.tile([128, 2, SEQ], BF16)  # k_projT
            vp = cst.tile([128, NKT, H * (DH + 1)], BF16)  # v_proj + ones cols
            ones_t = cst.tile([128, 64], BF16)
            nc.vector.memset(ones_t[:], 1.0)
            vrow1 = cst.tile([1, SEQ], BF16)
            nc.vector.memset(vrow1[:], 1.0)

            # Transient projection PSUM pieces borrow the rotating "sc"
            # ring (all its occupants are short-lived); the "po" ring is
            # reserved for the long-lived PV accumulators.
            def proj_k(m, s4):
                pk = ps.tile([128, 512], F32, tag="sc", bufs=3, name=f"pk{m}{s4}")
                for f in range(2):
                    nc.tensor.matmul(
                        pk[:],
                        wk_t[:, f, ts(m, 128)],
                        kT_t[:, f, ts(s4, 512)],
                        start=(f == 0),
                        stop=(f == 1),
                    )
                nc.vector.tensor_copy(kp[:, m, ts(s4, 512)], pk[:])

            def proj_q(m):
                pq = ps.tile([128, QB], F32, tag="sc", bufs=3, name=f"pq{m}")
                for f in range(2):
                    nc.tensor.matmul(
                        pq[:],
                        wq_t[:, f, ts(m, 128)],
                        qT_t[:, f, :],
                        start=(f == 0),
                        stop=(f == 1),
                    )
                nc.vector.tensor_copy(qp[:, m, :], pq[:])

            def proj_v(st):
                # third K=1 matmul of the host-side ones row against the
                # augmented Wv row produces the per-head ones columns, so
                # the PSUM->SBUF copy is fully contiguous (strided DVE
                # writes misbehave on HW).
                pv = ps.tile(
                    [128, H * (DH + 1)], F32, tag="sc", bufs=3, name=f"pv{st}"
                )
                for f in range(2):
                    nc.tensor.matmul(
                        pv[:],
                        vT_t[:, f, ts(st, 128)],
                        wv_t[:, f, :],
                        start=(f == 0),
                        stop=False,
                    )
                nc.tensor.matmul(
                    pv[:],
                    vrow1[0:1, ts(st, 128)],
                    wva_t[:],
                    start=False,
                    stop=True,
                )
                nc.vector.tensor_copy(vp[:, st, :], pv[:])

            # attn[(m, t)] = exp(scoresT) for head pair t of quad m:
            # [k-chunk part, ct, r', q]
            attn = {}
            for m in range(2):
                for t in range(2):
                    attn[(m, t)] = sb.tile(
                        [128, NKT, 2, 512], BF16, tag="attn", bufs=4,
                        name=f"attn{m}{t}",
                    )

            def qk_fill(m, t, ct):
                slot = ps.tile(
                    [128, 2, 512], F32, tag="sc", bufs=3, name=f"sc{m}{t}{ct}"
                )
                for rr in range(2):
                    r = 2 * t + rr
                    nc.tensor.matmul(
                        slot[:, rr, :],
                        kp[ts(r, 32), m, ts(ct, 128)],
                        qp[ts(r, 32), m, :],
                        start=True,
                        stop=True,
                        tile_position=(32 * r, 0),
                    )
                return slot

            def qk_act(m, t, ct, slot):
                nc.scalar.activation(attn[(m, t)][:, ct, :, :], slot[:], EXP)

            po = {}  # (m, t) -> PSUM accumulator [128, 512]

            def pv_pair(m, t, ct):
                """PV for both heads of pair (m,t), k-chunk ct.  The two
                matmuls sit at col positions 0/64 so they run concurrently."""
                if ct == 0:
                    po[(m, t)] = ps.tile(
                        [128, 512], F32, tag="po", bufs=2, name=f"po{m}{t}"
                    )
                p = po[(m, t)]
                for rr in range(2):
                    h = 4 * m + 2 * t + rr
                    base = 64 * rr
                    nc.tensor.matmul(
                        p[base : base + DH + 1, :],
                        vp[:, ct, ts(h, DH + 1)],
                        attn[(m, t)][:, ct, rr, :],
                        start=(ct == 0),
                        stop=(ct == NKT - 1),
                        tile_position=(0, base),
                        skip_group_check=True,
                    )

            prod = {}

            def stage_c(m, t):
                """normalize: prod = po * (1 / PE-broadcast(denominator rows)).

                The denominator rows sit at partitions 32/96 of each PV
                accumulator; a K=1 matmul against a ones sliver replicates
                each across its head's 32 output partitions (DVE/ACT cannot
                move data across partitions)."""
                p = po[(m, t)]
                dsb = sb.tile([128, 512], BF16, tag="dsb", bufs=2, name=f"dsb{m}{t}")
                bc = ps.tile([128, 512], F32, tag="sc", bufs=3, name=f"bc{m}{t}")
                rsb = sb.tile([128, 512], F32, tag="rsb", bufs=2, name=f"rsb{m}{t}")
                prod[(m, t)] = sb.tile(
                    [128, 512], BF16, tag="prod", bufs=4, name=f"prod{m}{t}"
                )
                for base in (0, 64):
                    nc.vector.tensor_copy(
                        dsb[base + DH : base + DH + 1, :],
                        p[base + DH : base + DH + 1, :],
                    )
                    # M=64 fills bc completely so the full-tile reciprocal
                    # below reads no stale slot bytes.
                    nc.tensor.matmul(
                        bc[base : base + 64, :],
                        ones_t[base + DH : base + DH + 1, :],
                        dsb[base + DH : base + DH + 1, :],
                        start=True,
                        stop=True,
                        tile_position=(base + DH, base),
                        skip_group_check=True,
                    )
                nc.vector.reciprocal_approx_fast(rsb[:], bc[:])
                for base in (0, 64):
                    nc.vector.tensor_mul(
                        prod[(m, t)][base : base + DH, :],
                        p[base : base + DH, :],
                        rsb[base : base + DH, :],
                    )

            # ================= schedule =================
            # startup: minimal prefix for the first score slots.
            proj_k(0, 0)
            proj_q(0)

            def PV(m, t, ct):
                return lambda: pv_pair(m, t, ct)

            def SC(m, t):
                return lambda: stage_c(m, t)

            def PK(m, s4):
                return lambda: proj_k(m, s4)

            def PVJ(st):
                return lambda: proj_v(st)

            # per-phase work items: {slot_index: [callables]} + drain list.
            # PE load per slot is kept at or below the ~1us ScalarE drain.
            p1 = {
                0: [PK(0, 1)], 1: [PK(0, 2)], 2: [PK(0, 3)],
                3: [lambda: proj_q(1)],
                4: [PK(1, 0)], 5: [PK(1, 1)], 6: [PK(1, 2)], 7: [PK(1, 3)],
            }
            for i in range(8):
                p1[8 + i] = [PVJ(i)]
            p2 = {i: [PVJ(8 + i)] for i in range(8)}
            for i in range(NKT):
                p2.setdefault(i, []).append(PV(0, 0, i))
            p3 = {0: [SC(0, 0)]}
            for i in range(NKT):
                p3.setdefault(i, []).append(PV(0, 1, i))
            for i in range(2, NKT):
                p3.setdefault(i, []).append(PV(1, 0, i - 2))  # ct 0..13
            p4 = {
                0: [PV(1, 0, 14), SC(0, 1)],
                1: [PV(1, 0, 15), SC(1, 0)],
            }
            for i in range(2, NKT):
                p4.setdefault(i, []).append(PV(1, 1, i - 2))  # ct 0..13
            phases = [
                ((0, 0), p1, []),
                ((0, 1), p2, []),
                ((1, 0), p3, []),
                ((1, 1), p4, [PV(1, 1, 14), PV(1, 1, 15)]),
            ]

            LOOKAHEAD = 2
            for (m, t), items, drain in phases:
                slots = {}
                for ct in range(LOOKAHEAD):
                    slots[ct] = qk_fill(m, t, ct)
                for ct in range(NKT):
                    if ct + LOOKAHEAD < NKT:
                        slots[ct + LOOKAHEAD] = qk_fill(m, t, ct + LOOKAHEAD)
                    qk_act(m, t, ct, slots.pop(ct))
                    for w in items.get(ct, []):
                        w()
                for w in drain:
                    w()

            stage_c(1, 1)

            # ---- final projection outT = Wo @ concatT + bo, as K=32
            # matmuls per (quad, pv-tile, half) against the permuted WoT.
            # base 0/64 groups alternate so they row-pack on the PE.
            out_sb = cst.tile([128, 2, QB], F32)
            pf = {}
            for o in range(2):
                for base in (0, 64):
                    tag, bufs = ("sc", 3) if o == 0 else ("po", 2)
                    pf[(o, base)] = ps.tile(
                        [128, QB], F32, tag=tag, bufs=bufs, name=f"pf{o}{base}"
                    )
            for idx, (m, t) in enumerate([(0, 0), (0, 1), (1, 0), (1, 1)]):
                for o in range(2):
                    for base in (0, 64):
                        nc.tensor.matmul(
                            pf[(o, base)][:],
                            wo_t[base : base + DH, 2 * m + t, ts(o, 128)],
                            prod[(m, t)][base : base + DH, :],
                            start=(idx == 0),
                            stop=(idx == 3),
                            tile_position=(base, 0),
                            skip_group_check=True,
                        )
            for o in range(2):
                nc.vector.tensor_scalar_add(
                    out_sb[:, o, :], pf[(o, 0)][:], bo_t[:, o, :]
                )
                nc.vector.tensor_add(out_sb[:, o, :], out_sb[:, o, :], pf[(o, 64)][:])
                nc.sync.dma_start(outT[ts(o, 128), :], out_sb[:, o, :])

    nc.compile()
    return nc


_NC = None


def _get_nc():
    global _NC
    if _NC is None:
        _NC = _build_graph()
    return _NC


def prep_in_maps(query, key, value, Wq, Wk, Wv, Wo, bo):
    bf = ml_dtypes.bfloat16
    scale = np.float32(1.0 / np.sqrt(DH))

    wqT = np.ascontiguousarray((Wq.astype(np.float32) * scale).T).astype(bf)
    wkT = np.ascontiguousarray(Wk.T).astype(bf)
    # augmented WvT: [257 in-feats (last = ones row), 8 heads x 33 out-cols]
    wvT_a = np.zeros((D + 1, H * (DH + 1)), np.float32)
    wvt = Wv.T.astype(np.float32)  # [in 256, out 256]
    for h in range(H):
        wvT_a[:D, (DH + 1) * h : (DH + 1) * h + DH] = wvt[:, DH * h : DH * (h + 1)]
        wvT_a[D, (DH + 1) * h + DH] = 1.0
    wvT = np.ascontiguousarray(wvT_a).astype(bf)
    # permuted WoT: head h = 4m + 2t + rr lives at partition rows
    # 64*rr .. +32 of free-block 2m+t, matching PV output partitions.
    woP = np.zeros((128, 4, D), np.float32)
    woT = Wo.T.astype(np.float32)  # [hd, out]
    for h in range(H):
        m, r = h // 4, h % 4
        blk, base = 2 * m + r // 2, 64 * (r % 2)
        woP[base : base + DH, blk, :] = woT[DH * h : DH * (h + 1), :]
    woP = np.ascontiguousarray(woP.reshape(128, 4 * D)).astype(bf)
    bo_c = np.ascontiguousarray(bo.astype(np.float32).reshape(D, 1))

    kT_b = [np.ascontiguousarray(key[b].T).astype(bf) for b in range(B)]
    vT_b = [np.ascontiguousarray(value[b].T).astype(bf) for b in range(B)]

    in_maps = []
    for c in range(8):
        b, qb = c // 4, c % 4
        in_maps.append(
            {
                "qT": np.ascontiguousarray(
                    query[b, qb * QB : (qb + 1) * QB, :].T
                ).astype(bf),
                "kT": kT_b[b],
                "vT": vT_b[b],
                "wqT": wqT,
                "wkT": wkT,
                "wvT": wvT,
                "woP": woP,
                "bo": bo_c,
            }
        )
    return in_maps


def kernel(query, key, value, Wq, Wk, Wv, Wo, bo):
    nc = _get_nc()
    in_maps = prep_in_maps(query, key, value, Wq, Wk, Wv, Wo, bo)
    res = run_bass_kernel_spmd(nc, in_maps, core_ids=list(range(8)))

    out = np.empty((B, SEQ, D), np.float32)
    for c in range(8):
        b, qb = c // 4, c % 4
        out[b, qb * QB : (qb + 1) * QB, :] = res.results[c]["outT"].T
    return out


# revision 11
# speedup vs baseline: 1.0108x; 1.0108x over previous
"""Multi-head attention (B=2, S=2048, H=8, Dh=32, D=256) on 8 TRN2 NeuronCores.

Sharding: core c -> (batch b = c//4, query-block qb = c%4 of 512 rows).
Each core computes full attention + output projection for its 512 query rows;
no cross-core communication is needed.  Host does layout prep only
(transposes + bf16 casts); all FLOPs run on device.

Device-side layout (per core):
  - raw activations shipped transposed: qT [256f, 512q], kT/vT [256f, 2048s]
  - q/k projections produced directly transposed (head-dim on partitions,
    head h at partitions 32*(h%4) of free-block h//4) so QK^T runs as
    K=32 row-tiled matmuls; the two heads of a pair are issued
    back-to-back at row positions 32r so the PE runs them concurrently.
  - scores computed TRANSPOSED: scoresT[k, q]; exp is a straight ScalarE
    pass over PSUM (no max subtraction: scores ~ N(0,1)).
  - v projected to natural layout augmented with a ones column per head
    ([128, 16, 8, 33]) so each PV matmul (M=33) also accumulates the
    softmax denominator as its last output row - no separate reduction.
  - normalization: reciprocal of the two denominator rows (partitions 32
    and 96) + K=1 matmul row-broadcast + one DVE multiply per band.
  - final projection: K=32 matmuls per head slice against a host-permuted
    WoT whose row bands match the PV output partition bases.

Schedule (v2): ScalarE is the target critical path (~8.4M exps/core at
1 elem/lane/cycle + ~170cyc/instr overhead -> ~64us).  Its stream carries
ONLY the exp ACTIVATEs (DMA triggers live on sync/vector/gpsimd queues).
Exp regions are [128, 2(heads), 512] PSUM slots (one k-chunk x head-pair,
N=1024); QK production runs 2 slots ahead.  Projections (phase 1-2) and
PV/normalize (phases 2-4) are spread between QK groups so each phase's PE
load roughly matches the ScalarE drain rate, keeping the PE stream dense
(HAM stays released) without starving exp.  The output projection runs in
the tail on freed PSUM slots.  PSUM budget: 3x[128,2,512] rotating slots
(scores + transients + broadcast + final o=0) and 2x[128,512] accumulator
slots (PV quads in flight + final o=1) = exactly 8 banks.
"""

import sys

sys.path.insert(0, "/opt/trn_rl_repo")

import numpy as np
import ml_dtypes

import concourse.bass as bass
import concourse.bacc as bacc
import concourse.mybir as mybir
from concourse.tile import TileContext
from concourse.bass import ts
from concourse.bass_utils import run_bass_kernel_spmd

BF16 = mybir.dt.bfloat16
F32 = mybir.dt.float32
EXP = mybir.ActivationFunctionType.Exp

B, SEQ, D = 2, 2048, 256
H, DH = 8, 32
QB = 512  # query rows per core
NKT = SEQ // 128  # 16 k-chunk tiles (partition tiles of scoresT)


def _build_graph():
    nc = bacc.Bacc("TRN2", target_bir_lowering=False, debug=False)

    qT = nc.declare_dram_parameter("qT", [D, QB], BF16, isOutput=False)
    kT = nc.declare_dram_parameter("kT", [D, SEQ], BF16, isOutput=False)
    vT = nc.declare_dram_parameter("vT", [D, SEQ], BF16, isOutput=False)
    wqT = nc.declare_dram_parameter("wqT", [D, D], BF16, isOutput=False)
    wkT = nc.declare_dram_parameter("wkT", [D, D], BF16, isOutput=False)
    wvT = nc.declare_dram_parameter("wvT", [D + 1, H * (DH + 1)], BF16, isOutput=False)
    woP = nc.declare_dram_parameter("woP", [128, 4 * D], BF16, isOutput=False)
    bo = nc.declare_dram_parameter("bo", [D, 1], F32, isOutput=False)
    outT = nc.declare_dram_parameter("outT", [D, QB], F32, isOutput=True)

    with TileContext(nc) as tc:
        with (
            tc.tile_pool(name="cst", bufs=1) as cst,
            tc.tile_pool(name="sb", bufs=1) as sb,
            tc.tile_pool(name="ps", bufs=2, space="PSUM") as ps,
        ):
            # warm loads the exp table set early so the ~2.7us
            # ACT_TABLE_LOAD overlaps the DMA/projection phase.
            warm = cst.tile([1, 1], F32)
            nc.vector.memset(warm[:], 0.0)
            nc.scalar.activation(warm[:], warm[:], EXP)

            # ---- inputs.  Everything on the sync HWDGE queue in priority
            # order (k/q path first, v chunks interleaved behind the k
            # chunks they pace with); only the tail-needed wo/bo ride the
            # slow gpsimd SWDGE queue.  NOTHING on the scalar queue - it
            # must only carry exps.
            wk_t = cst.tile([128, 2, D], BF16)
            wq_t = cst.tile([128, 2, D], BF16)
            kT_t = cst.tile([128, 2, SEQ], BF16)
            qT_t = cst.tile([128, 2, QB], BF16)
            wv_t = cst.tile([128, 2, H * (DH + 1)], BF16)
            wva_t = cst.tile([1, H * (DH + 1)], BF16)
            vT_t = cst.tile([128, 2, SEQ], BF16)
            wo_t = cst.tile([128, 4, D], BF16)
            bo_t = cst.tile([128, 2, 1], F32)

            for f in range(2):
                nc.sync.dma_start(wk_t[:, f, :], wkT[ts(f, 128), :])
            for f in range(2):
                nc.sync.dma_start(wq_t[:, f, :], wqT[ts(f, 128), :])
                nc.sync.dma_start(qT_t[:, f, :], qT[ts(f, 128), :])
            for f in range(2):
                nc.sync.dma_start(
                    kT_t[:, f, ts(0, 512)], kT[ts(f, 128), ts(0, 512)]
                )
            for f in range(2):
                nc.sync.dma_start(wv_t[:, f, :], wvT[ts(f, 128), :])
            nc.sync.dma_start(wva_t[:], wvT[D : D + 1, :])
            for s4 in range(4):
                if s4 < 3:
                    for f in range(2):
                        nc.sync.dma_start(
                            kT_t[:, f, ts(s4 + 1, 512)],
                            kT[ts(f, 128), ts(s4 + 1, 512)],
                        )
                for f in range(2):
                    nc.sync.dma_start(
                        vT_t[:, f, ts(s4, 512)], vT[ts(f, 128), ts(s4, 512)]
                    )
            nc.gpsimd.dma_start(wo_t[:, :, :], woP.rearrange("p (b d) -> p b d", b=4))
            for o in range(2):
                nc.gpsimd.dma_start(bo_t[:, o, :], bo[ts(o, 128), :])

            # ---- SBUF destinations for projections.
            qp = cst.tile([128, 2, QB], BF16)  # q_projT
            kp = cst.tile([128, 2, SEQ], BF16)  # k_projT
            vp = cst.tile([128, NKT, H * (DH + 1)], BF16)  # v_proj + ones cols
            ones_t = cst.tile([128, 64], BF16)
            nc.vector.memset(ones_t[:], 1.0)
            vrow1 = cst.tile([1, SEQ], BF16)
            nc.vector.memset(vrow1[:], 1.0)

            # ---- PE warm-up burst.  The HAM clock gate releases (1.2 ->
            # 2.4 GHz) only after ~3.4us of gapless PE activity; the DMA
            # ramp leaves the PE idle exactly that long, so burn it with
            # back-to-back dummy matmuls (no cross-engine deps between
            # them - same-engine program order keeps the stream dense).
            dmy = cst.tile([128, 512], BF16)
            nc.vector.memset(dmy[:], 0.0)
            pdmy = ps.tile([128, 512], F32, tag="po", bufs=2, name="pdmy")
            for i in range(9):
                nc.tensor.matmul(
                    pdmy[:64, :],
                    ones_t[:, :],
                    dmy[:, :],
                    start=True,
                    stop=True,
                )

            # Transient projection PSUM pieces borrow the rotating "sc"
            # ring (all its occupants are short-lived); the "po" ring is
            # reserved for the long-lived PV accumulators.
            def proj_k(m, s4):
                pk = ps.tile([128, 512], F32, tag="sc", bufs=3, name=f"pk{m}{s4}")
                for f in range(2):
                    nc.tensor.matmul(
                        pk[:],
                        wk_t[:, f, ts(m, 128)],
                        kT_t[:, f, ts(s4, 512)],
                        start=(f == 0),
                        stop=(f == 1),
                    )
                nc.vector.tensor_copy(kp[:, m, ts(s4, 512)], pk[:])

            def proj_q(m):
                pq = ps.tile([128, QB], F32, tag="sc", bufs=3, name=f"pq{m}")
                for f in range(2):
                    nc.tensor.matmul(
                        pq[:],
                        wq_t[:, f, ts(m, 128)],
                        qT_t[:, f, :],
                        start=(f == 0),
                        stop=(f == 1),
                    )
                nc.vector.tensor_copy(qp[:, m, :], pq[:])

            def proj_v(st):
                # third K=1 matmul of the host-side ones row against the
                # augmented Wv row produces the per-head ones columns, so
                # the PSUM->SBUF copy is fully contiguous (strided DVE
                # writes misbehave on HW).
                pv = ps.tile(
                    [128, H * (DH + 1)], F32, tag="sc", bufs=3, name=f"pv{st}"
                )
                for f in range(2):
                    nc.tensor.matmul(
                        pv[:],
                        vT_t[:, f, ts(st, 128)],
                        wv_t[:, f, :],
                        start=(f == 0),
                        stop=False,
                    )
                nc.tensor.matmul(
                    pv[:],
                    vrow1[0:1, ts(st, 128)],
                    wva_t[:],
                    start=False,
                    stop=True,
                )
                nc.vector.tensor_copy(vp[:, st, :], pv[:])

            # attn[(m, t)] = exp(scoresT) for head pair t of quad m:
            # [k-chunk part, ct, r', q]
            attn = {}
            for m in range(2):
                for t in range(2):
                    attn[(m, t)] = sb.tile(
                        [128, NKT, 2, 512], BF16, tag="attn", bufs=4,
                        name=f"attn{m}{t}",
                    )

            def qk_fill(m, t, ct):
                slot = ps.tile(
                    [128, 2, 512], F32, tag="sc", bufs=3, name=f"sc{m}{t}{ct}"
                )
                for rr in range(2):
                    r = 2 * t + rr
                    nc.tensor.matmul(
                        slot[:, rr, :],
                        kp[ts(r, 32), m, ts(ct, 128)],
                        qp[ts(r, 32), m, :],
                        start=True,
                        stop=True,
                        tile_position=(32 * r, 0),
                    )
                return slot

            def qk_act(m, t, ct, slot):
                nc.scalar.activation(attn[(m, t)][:, ct, :, :], slot[:], EXP)

            po = {}  # (m, t) -> PSUM accumulator [128, 512]

            def pv_pair(m, t, ct):
                """PV for both heads of pair (m,t), k-chunk ct.  The two
                matmuls sit at col positions 0/64 so they run concurrently."""
                if ct == 0:
                    po[(m, t)] = ps.tile(
                        [128, 512], F32, tag="po", bufs=2, name=f"po{m}{t}"
                    )
                p = po[(m, t)]
                for rr in range(2):
                    h = 4 * m + 2 * t + rr
                    base = 64 * rr
                    nc.tensor.matmul(
                        p[base : base + DH + 1, :],
                        vp[:, ct, ts(h, DH + 1)],
                        attn[(m, t)][:, ct, rr, :],
                        start=(ct == 0),
                        stop=(ct == NKT - 1),
                        tile_position=(0, base),
                        skip_group_check=True,
                    )

            prod = {}

            def stage_c(m, t):
                """normalize: prod = po * (1 / PE-broadcast(denominator rows)).

                The denominator rows sit at partitions 32/96 of each PV
                accumulator; a K=1 matmul against a ones sliver replicates
                each across its head's 32 output partitions (DVE/ACT cannot
                move data across partitions)."""
                p = po[(m, t)]
                dsb = sb.tile([128, 512], BF16, tag="dsb", bufs=2, name=f"dsb{m}{t}")
                bc = ps.tile([128, 512], F32, tag="sc", bufs=3, name=f"bc{m}{t}")
                rsb = sb.tile([128, 512], F32, tag="rsb", bufs=2, name=f"rsb{m}{t}")
                prod[(m, t)] = sb.tile(
                    [128, 512], BF16, tag="prod", bufs=4, name=f"prod{m}{t}"
                )
                for base in (0, 64):
                    nc.vector.tensor_copy(
                        dsb[base + DH : base + DH + 1, :],
                        p[base + DH : base + DH + 1, :],
                    )
                    # M=64 fills bc completely so the full-tile reciprocal
                    # below reads no stale slot bytes.
                    nc.tensor.matmul(
                        bc[base : base + 64, :],
                        ones_t[base + DH : base + DH + 1, :],
                        dsb[base + DH : base + DH + 1, :],
                        start=True,
                        stop=True,
                        tile_position=(base + DH, base),
                        skip_group_check=True,
                    )
                nc.vector.reciprocal_approx_fast(rsb[:], bc[:])
                for base in (0, 64):
                    nc.vector.tensor_mul(
                        prod[(m, t)][base : base + DH, :],
                        p[base : base + DH, :],
                        rsb[base : base + DH, :],
                    )

            # ================= schedule =================
            # startup: minimal prefix for the first score slots.
            proj_k(0, 0)
            proj_q(0)

            def PV(m, t, ct):
                return lambda: pv_pair(m, t, ct)

            def SC(m, t):
                return lambda: stage_c(m, t)

            def PK(m, s4):
                return lambda: proj_k(m, s4)

            def PVJ(st):
                return lambda: proj_v(st)

            # per-phase work items: {slot_index: [callables]} + drain list.
            # PE load per slot is kept at or below the ~1us ScalarE drain.
            p1 = {
                0: [PK(0, 1)], 1: [PK(0, 2)], 2: [PK(0, 3)],
                3: [lambda: proj_q(1)],
                4: [PK(1, 0)], 5: [PK(1, 1)], 6: [PK(1, 2)], 7: [PK(1, 3)],
            }
            for i in range(8):
                p1[8 + i] = [PVJ(i)]
            p2 = {i: [PVJ(8 + i)] for i in range(8)}
            for i in range(NKT):
                p2.setdefault(i, []).append(PV(0, 0, i))
            p3 = {0: [SC(0, 0)]}
            for i in range(NKT):
                p3.setdefault(i, []).append(PV(0, 1, i))
            for i in range(3, NKT):
                p3.setdefault(i, []).append(PV(1, 0, i - 3))  # ct 0..12
            p4 = {
                0: [PV(1, 0, 13), SC(0, 1)],
                1: [PV(1, 0, 14)],
                2: [PV(1, 0, 15)],
                3: [SC(1, 0)],
            }
            for i in range(3, NKT):
                p4.setdefault(i, []).append(PV(1, 1, i - 3))  # ct 0..12
            phases = [
                ((0, 0), p1, []),
                ((0, 1), p2, []),
                ((1, 0), p3, []),
                ((1, 1), p4, [PV(1, 1, 13), PV(1, 1, 14), PV(1, 1, 15)]),
            ]

            LOOKAHEAD = 2
            for (m, t), items, drain in phases:
                slots = {}
                for ct in range(LOOKAHEAD):
                    slots[ct] = qk_fill(m, t, ct)
                for ct in range(NKT):
                    if ct + LOOKAHEAD < NKT:
                        slots[ct + LOOKAHEAD] = qk_fill(m, t, ct + LOOKAHEAD)
                    qk_act(m, t, ct, slots.pop(ct))
                    for w in items.get(ct, []):
                        w()
                for w in drain:
                    w()

            stage_c(1, 1)

            # ---- final projection outT = Wo @ concatT + bo, as K=32
            # matmuls per (quad, pv-tile, half) against the permuted WoT.
            # base 0/64 groups alternate so they row-pack on the PE.
            out_sb = cst.tile([128, 2, QB], F32)
            pf = {}
            for o in range(2):
                for base in (0, 64):
                    tag, bufs = ("sc", 3) if o == 0 else ("po", 2)
                    pf[(o, base)] = ps.tile(
                        [128, QB], F32, tag=tag, bufs=bufs, name=f"pf{o}{base}"
                    )
            # o=0 fully first so its DVE combine + store overlap o=1's MMs.
            for o in range(2):
                for idx, (m, t) in enumerate([(0, 0), (0, 1), (1, 0), (1, 1)]):
                    for base in (0, 64):
                        nc.tensor.matmul(
                            pf[(o, base)][:],
                            wo_t[base : base + DH, 2 * m + t, ts(o, 128)],
                            prod[(m, t)][base : base + DH, :],
                            start=(idx == 0),
                            stop=(idx == 3),
                            tile_position=(base, 0),
                            skip_group_check=True,
                        )
                nc.vector.tensor_scalar_add(
                    out_sb[:, o, :], pf[(o, 0)][:], bo_t[:, o, :]
                )
                nc.vector.tensor_add(
                    out_sb[:, o, :], out_sb[:, o, :], pf[(o, 64)][:]
                )
                nc.sync.dma_start(outT[ts(o, 128), :], out_sb[:, o, :])

    nc.compile()
    return nc


_NC = None


def _get_nc():
    global _NC
    if _NC is None:
        _NC = _build_graph()
    return _NC


def prep_in_maps(query, key, value, Wq, Wk, Wv, Wo, bo):
    bf = ml_dtypes.bfloat16
    scale = np.float32(1.0 / np.sqrt(DH))

    wqT = np.ascontiguousarray((Wq.astype(np.float32) * scale).T).astype(bf)
    wkT = np.ascontiguousarray(Wk.T).astype(bf)
    # augmented WvT: [257 in-feats (last = ones row), 8 heads x 33 out-cols]
    wvT_a = np.zeros((D + 1, H * (DH + 1)), np.float32)
    wvt = Wv.T.astype(np.float32)  # [in 256, out 256]
    for h in range(H):
        wvT_a[:D, (DH + 1) * h : (DH + 1) * h + DH] = wvt[:, DH * h : DH * (h + 1)]
        wvT_a[D, (DH + 1) * h + DH] = 1.0
    wvT = np.ascontiguousarray(wvT_a).astype(bf)
    # permuted WoT: head h = 4m + 2t + rr lives at partition rows
    # 64*rr .. +32 of free-block 2m+t, matching PV output partitions.
    woP = np.zeros((128, 4, D), np.float32)
    woT = Wo.T.astype(np.float32)  # [hd, out]
    for h in range(H):
        m, r = h // 4, h % 4
        blk, base = 2 * m + r // 2, 64 * (r % 2)
        woP[base : base + DH, blk, :] = woT[DH * h : DH * (h + 1), :]
    woP = np.ascontiguousarray(woP.reshape(128, 4 * D)).astype(bf)
    bo_c = np.ascontiguousarray(bo.astype(np.float32).reshape(D, 1))

    kT_b = [np.ascontiguousarray(key[b].T).astype(bf) for b in range(B)]
    vT_b = [np.ascontiguousarray(value[b].T).astype(bf) for b in range(B)]

    in_maps = []
    for c in range(8):
        b, qb = c // 4, c % 4
        in_maps.append(
            {
                "qT": np.ascontiguousarray(
                    query[b, qb * QB : (qb + 1) * QB, :].T
                ).astype(bf),
                "kT": kT_b[b],
                "vT": vT_b[b],
                "wqT": wqT,
                "wkT": wkT,
                "wvT": wvT,
                "woP": woP,
                "bo": bo_c,
            }
        )
    return in_maps


def kernel(query, key, value, Wq, Wk, Wv, Wo, bo):
    nc = _get_nc()
    in_maps = prep_in_maps(query, key, value, Wq, Wk, Wv, Wo, bo)
    res = run_bass_kernel_spmd(nc, in_maps, core_ids=list(range(8)))

    out = np.empty((B, SEQ, D), np.float32)
    for c in range(8):
        b, qb = c // 4, c % 4
        out[b, qb * QB : (qb + 1) * QB, :] = res.results[c]["outT"].T
    return out


# revision 13
# speedup vs baseline: 1.1943x; 1.1816x over previous
"""Multi-head attention (B=2, S=2048, H=8, Dh=32, D=256) on 8 TRN2 NeuronCores.

Sharding: core c -> (batch b = c//4, query-block qb = c%4 of 512 rows).
Each core computes full attention + output projection for its 512 query rows;
no cross-core communication is needed.  Host does layout prep only
(transposes + bf16 casts); all FLOPs run on device.

Device-side layout (per core):
  - raw activations shipped transposed: qT [256f, 512q], kT/vT [256f, 2048s]
  - q/k projections produced directly transposed (head-dim on partitions,
    head h at partitions 32*(h%4) of free-block h//4) so QK^T runs as
    K=32 row-tiled matmuls; the two heads of a pair are issued
    back-to-back at row positions 32r so the PE runs them concurrently.
  - scores computed TRANSPOSED: scoresT[k, q]; exp is a straight ScalarE
    pass over PSUM (no max subtraction: scores ~ N(0,1)).
  - v projected to natural layout augmented with a ones column per head
    ([128, 16, 8, 33]) so each PV matmul (M=33) also accumulates the
    softmax denominator as its last output row - no separate reduction.
  - normalization: reciprocal of the two denominator rows (partitions 32
    and 96) + K=1 matmul row-broadcast + one DVE multiply per band.
  - final projection: K=32 matmuls per head slice against a host-permuted
    WoT whose row bands match the PV output partition bases.

Schedule (v2): ScalarE is the target critical path (~8.4M exps/core at
1 elem/lane/cycle + ~170cyc/instr overhead -> ~64us).  Its stream carries
ONLY the exp ACTIVATEs (DMA triggers live on sync/vector/gpsimd queues).
Exp regions are [128, 2(heads), 512] PSUM slots (one k-chunk x head-pair,
N=1024); QK production runs 2 slots ahead.  Projections (phase 1-2) and
PV/normalize (phases 2-4) are spread between QK groups so each phase's PE
load roughly matches the ScalarE drain rate, keeping the PE stream dense
(HAM stays released) without starving exp.  The output projection runs in
the tail on freed PSUM slots.  PSUM budget: 3x[128,2,512] rotating slots
(scores + transients + broadcast + final o=0) and 2x[128,512] accumulator
slots (PV quads in flight + final o=1) = exactly 8 banks.
"""

import sys

sys.path.insert(0, "/opt/trn_rl_repo")

import numpy as np
import ml_dtypes

import concourse.bass as bass
import concourse.bacc as bacc
import concourse.mybir as mybir
from concourse.tile import TileContext
from concourse.bass import ts
from concourse.bass_utils import run_bass_kernel_spmd

BF16 = mybir.dt.bfloat16
F32 = mybir.dt.float32
EXP = mybir.ActivationFunctionType.Exp

B, SEQ, D = 2, 2048, 256
H, DH = 8, 32
QB = 512  # query rows per core
NKT = SEQ // 128  # 16 k-chunk tiles (partition tiles of scoresT)


def _build_graph():
    nc = bacc.Bacc("TRN2", target_bir_lowering=False, debug=False)

    qT = nc.declare_dram_parameter("qT", [D, QB], BF16, isOutput=False)
    kT = nc.declare_dram_parameter("kT", [D, SEQ], BF16, isOutput=False)
    vT = nc.declare_dram_parameter("vT", [D, SEQ], BF16, isOutput=False)
    wqT = nc.declare_dram_parameter("wqT", [D, D], BF16, isOutput=False)
    wkT = nc.declare_dram_parameter("wkT", [D, D], BF16, isOutput=False)
    wvT = nc.declare_dram_parameter("wvT", [D + 1, H * (DH + 1)], BF16, isOutput=False)
    woP = nc.declare_dram_parameter("woP", [128, 4 * D], BF16, isOutput=False)
    bo = nc.declare_dram_parameter("bo", [D, 1], F32, isOutput=False)
    outT = nc.declare_dram_parameter("outT", [D, QB], F32, isOutput=True)

    with TileContext(nc) as tc:
        with (
            tc.tile_pool(name="cst", bufs=1) as cst,
            tc.tile_pool(name="sb", bufs=1) as sb,
            tc.tile_pool(name="ps", bufs=2, space="PSUM") as ps,
        ):
            # warm loads the exp table set early so the ~2.7us
            # ACT_TABLE_LOAD overlaps the DMA/projection phase.
            warm = cst.tile([1, 1], F32)
            nc.vector.memset(warm[:], 0.0)
            nc.scalar.activation(warm[:], warm[:], EXP)

            # ---- inputs.  Everything on the sync HWDGE queue in priority
            # order (k/q path first, v chunks interleaved behind the k
            # chunks they pace with); only the tail-needed wo/bo ride the
            # slow gpsimd SWDGE queue.  NOTHING on the scalar queue - it
            # must only carry exps.
            wk_t = cst.tile([128, 2, D], BF16)
            wq_t = cst.tile([128, 2, D], BF16)
            kT_t = cst.tile([128, 2, SEQ], BF16)
            qT_t = cst.tile([128, 2, QB], BF16)
            wv_t = cst.tile([128, 2, H * (DH + 1)], BF16)
            wva_t = cst.tile([1, H * (DH + 1)], BF16)
            vT_t = cst.tile([128, 2, SEQ], BF16)
            wo_t = cst.tile([128, 4, D], BF16)
            bo_t = cst.tile([128, 2, 1], F32)

            for f in range(2):
                nc.sync.dma_start(wq_t[:, f, :], wqT[ts(f, 128), :])
            for f in range(2):
                nc.sync.dma_start(qT_t[:, f, :], qT[ts(f, 128), :])
            for f in range(2):
                nc.sync.dma_start(wk_t[:, f, :], wkT[ts(f, 128), :])
            for f in range(2):
                nc.sync.dma_start(kT_t[:, f, :], kT[ts(f, 128), :])
            for f in range(2):
                nc.sync.dma_start(vT_t[:, f, :], vT[ts(f, 128), :])
            for f in range(2):
                nc.sync.dma_start(wv_t[:, f, :], wvT[ts(f, 128), :])
            nc.sync.dma_start(wva_t[:], wvT[D : D + 1, :])
            nc.sync.dma_start(wo_t[:, :, :], woP.rearrange("p (b d) -> p b d", b=4))
            for o in range(2):
                nc.sync.dma_start(bo_t[:, o, :], bo[ts(o, 128), :])

            # ---- SBUF destinations for projections.
            qp = cst.tile([128, 2, QB], BF16)  # q_projT
            kp = cst.tile([128, 2, SEQ], BF16)  # k_projT
            vp = cst.tile([128, NKT, H * (DH + 1)], BF16)  # v_proj + ones cols
            ones_t = cst.tile([128, 64], BF16)
            nc.vector.memset(ones_t[:], 1.0)
            vrow1 = cst.tile([1, SEQ], BF16)
            nc.vector.memset(vrow1[:], 1.0)

            # ---- PE warm-up burst.  The HAM clock gate releases (1.2 ->
            # 2.4 GHz) only after ~3.4us of gapless PE activity; the DMA
            # ramp leaves the PE idle exactly that long, so burn it with
            # back-to-back dummy matmuls (no cross-engine deps between
            # them - same-engine program order keeps the stream dense).
            dmy = cst.tile([128, 512], BF16)
            nc.vector.memset(dmy[:], 0.0)
            pdmy = ps.tile([128, 512], F32, tag="po", bufs=2, name="pdmy")
            for i in range(9):
                nc.tensor.matmul(
                    pdmy[:64, :],
                    ones_t[:, :],
                    dmy[:, :],
                    start=True,
                    stop=True,
                )

            # Transient projection PSUM pieces borrow the rotating "sc"
            # ring (all its occupants are short-lived); the "po" ring is
            # reserved for the long-lived PV accumulators.
            def proj_k(m, s4):
                pk = ps.tile([128, 512], F32, tag="sc", bufs=3, name=f"pk{m}{s4}")
                for f in range(2):
                    nc.tensor.matmul(
                        pk[:],
                        wk_t[:, f, ts(m, 128)],
                        kT_t[:, f, ts(s4, 512)],
                        start=(f == 0),
                        stop=(f == 1),
                    )
                nc.vector.tensor_copy(kp[:, m, ts(s4, 512)], pk[:])

            def proj_q(m):
                pq = ps.tile([128, QB], F32, tag="sc", bufs=3, name=f"pq{m}")
                for f in range(2):
                    nc.tensor.matmul(
                        pq[:],
                        wq_t[:, f, ts(m, 128)],
                        qT_t[:, f, :],
                        start=(f == 0),
                        stop=(f == 1),
                    )
                nc.vector.tensor_copy(qp[:, m, :], pq[:])

            def proj_v(st):
                # third K=1 matmul of the host-side ones row against the
                # augmented Wv row produces the per-head ones columns, so
                # the PSUM->SBUF copy is fully contiguous (strided DVE
                # writes misbehave on HW).
                pv = ps.tile(
                    [128, H * (DH + 1)], F32, tag="sc", bufs=3, name=f"pv{st}"
                )
                for f in range(2):
                    nc.tensor.matmul(
                        pv[:],
                        vT_t[:, f, ts(st, 128)],
                        wv_t[:, f, :],
                        start=(f == 0),
                        stop=False,
                    )
                nc.tensor.matmul(
                    pv[:],
                    vrow1[0:1, ts(st, 128)],
                    wva_t[:],
                    start=False,
                    stop=True,
                )
                nc.vector.tensor_copy(vp[:, st, :], pv[:])

            # attn[(m, t)] = exp(scoresT) for head pair t of quad m:
            # [k-chunk part, ct, r', q]
            attn = {}
            for m in range(2):
                for t in range(2):
                    attn[(m, t)] = sb.tile(
                        [128, NKT, 2, 512], BF16, tag="attn", bufs=4,
                        name=f"attn{m}{t}",
                    )

            def qk_fill(m, t, ct):
                slot = ps.tile(
                    [128, 2, 512], F32, tag="sc", bufs=3, name=f"sc{m}{t}{ct}"
                )
                for rr in range(2):
                    r = 2 * t + rr
                    nc.tensor.matmul(
                        slot[:, rr, :],
                        kp[ts(r, 32), m, ts(ct, 128)],
                        qp[ts(r, 32), m, :],
                        start=True,
                        stop=True,
                        tile_position=(32 * r, 0),
                    )
                return slot

            def qk_act(m, t, ct, slot):
                nc.scalar.activation(attn[(m, t)][:, ct, :, :], slot[:], EXP)

            po = {}  # (m, t) -> PSUM accumulator [128, 512]

            def pv_pair(m, t, ct):
                """PV for both heads of pair (m,t), k-chunk ct.  The two
                matmuls sit at col positions 0/64 so they run concurrently."""
                if ct == 0:
                    po[(m, t)] = ps.tile(
                        [128, 512], F32, tag="po", bufs=2, name=f"po{m}{t}"
                    )
                p = po[(m, t)]
                for rr in range(2):
                    h = 4 * m + 2 * t + rr
                    base = 64 * rr
                    nc.tensor.matmul(
                        p[base : base + DH + 1, :],
                        vp[:, ct, ts(h, DH + 1)],
                        attn[(m, t)][:, ct, rr, :],
                        start=(ct == 0),
                        stop=(ct == NKT - 1),
                        tile_position=(0, base),
                        skip_group_check=True,
                    )

            prod = {}

            def stage_c(m, t):
                """normalize: prod = po * (1 / PE-broadcast(denominator rows)).

                The denominator rows sit at partitions 32/96 of each PV
                accumulator; a K=1 matmul against a ones sliver replicates
                each across its head's 32 output partitions (DVE/ACT cannot
                move data across partitions)."""
                p = po[(m, t)]
                dsb = sb.tile([128, 512], BF16, tag="dsb", bufs=2, name=f"dsb{m}{t}")
                bc = ps.tile([128, 512], F32, tag="sc", bufs=3, name=f"bc{m}{t}")
                rsb = sb.tile([128, 512], F32, tag="rsb", bufs=2, name=f"rsb{m}{t}")
                prod[(m, t)] = sb.tile(
                    [128, 512], BF16, tag="prod", bufs=4, name=f"prod{m}{t}"
                )
                for base in (0, 64):
                    nc.vector.tensor_copy(
                        dsb[base + DH : base + DH + 1, :],
                        p[base + DH : base + DH + 1, :],
                    )
                    # M=64 fills bc completely so the full-tile reciprocal
                    # below reads no stale slot bytes.
                    nc.tensor.matmul(
                        bc[base : base + 64, :],
                        ones_t[base + DH : base + DH + 1, :],
                        dsb[base + DH : base + DH + 1, :],
                        start=True,
                        stop=True,
                        tile_position=(base + DH, base),
                        skip_group_check=True,
                    )
                nc.vector.reciprocal_approx_fast(rsb[:], bc[:])
                for base in (0, 64):
                    nc.vector.tensor_mul(
                        prod[(m, t)][base : base + DH, :],
                        p[base : base + DH, :],
                        rsb[base : base + DH, :],
                    )

            # ================= schedule =================
            # startup: q-proj as soon as its (early) DMA lands, a second
            # dummy batch to bridge the PE gap until kT arrives (keeps the
            # HAM released), then the k-proj prefix for the first slots.
            proj_q(0)
            for i in range(6):
                nc.tensor.matmul(
                    pdmy[:64, :],
                    ones_t[:, :],
                    dmy[:, :],
                    start=True,
                    stop=True,
                )
            proj_k(0, 0)

            def PV(m, t, ct):
                return lambda: pv_pair(m, t, ct)

            def SC(m, t):
                return lambda: stage_c(m, t)

            def PK(m, s4):
                return lambda: proj_k(m, s4)

            def PVJ(st):
                return lambda: proj_v(st)

            # per-phase work items: {slot_index: [callables]} + drain list.
            # PE load per slot is kept at or below the ~1us ScalarE drain.
            p1 = {
                0: [PK(0, 1)], 1: [PK(0, 2)], 2: [PK(0, 3)],
                3: [lambda: proj_q(1)],
                4: [PK(1, 0)], 5: [PK(1, 1)], 6: [PK(1, 2)], 7: [PK(1, 3)],
            }
            for i in range(8):
                p1[8 + i] = [PVJ(i)]
            p2 = {i: [PVJ(8 + i)] for i in range(8)}
            for i in range(NKT):
                p2.setdefault(i, []).append(PV(0, 0, i))
            p3 = {0: [SC(0, 0)]}
            for i in range(NKT):
                p3.setdefault(i, []).append(PV(0, 1, i))
            for i in range(3, NKT):
                p3.setdefault(i, []).append(PV(1, 0, i - 3))  # ct 0..12
            p4 = {
                0: [PV(1, 0, 13), SC(0, 1)],
                1: [PV(1, 0, 14)],
                2: [PV(1, 0, 15)],
                3: [SC(1, 0)],
            }
            for i in range(3, NKT):
                p4.setdefault(i, []).append(PV(1, 1, i - 3))  # ct 0..12
            phases = [
                ((0, 0), p1, []),
                ((0, 1), p2, []),
                ((1, 0), p3, []),
                ((1, 1), p4, [PV(1, 1, 13), PV(1, 1, 14), PV(1, 1, 15)]),
            ]

            LOOKAHEAD = 2
            for (m, t), items, drain in phases:
                slots = {}
                for ct in range(LOOKAHEAD):
                    slots[ct] = qk_fill(m, t, ct)
                for ct in range(NKT):
                    if ct + LOOKAHEAD < NKT:
                        slots[ct + LOOKAHEAD] = qk_fill(m, t, ct + LOOKAHEAD)
                    qk_act(m, t, ct, slots.pop(ct))
                    for w in items.get(ct, []):
                        w()
                for w in drain:
                    w()

            stage_c(1, 1)

            # ---- final projection outT = Wo @ concatT + bo, as K=32
            # matmuls per (quad, pv-tile, half) against the permuted WoT.
            # base 0/64 groups alternate so they row-pack on the PE.
            out_sb = cst.tile([128, 2, QB], F32)
            pf = {}
            for o in range(2):
                for base in (0, 64):
                    tag, bufs = ("sc", 3) if o == 0 else ("po", 2)
                    pf[(o, base)] = ps.tile(
                        [128, QB], F32, tag=tag, bufs=bufs, name=f"pf{o}{base}"
                    )
            # o=0 fully first so its DVE combine + store overlap o=1's MMs.
            for o in range(2):
                for idx, (m, t) in enumerate([(0, 0), (0, 1), (1, 0), (1, 1)]):
                    for base in (0, 64):
                        nc.tensor.matmul(
                            pf[(o, base)][:],
                            wo_t[base : base + DH, 2 * m + t, ts(o, 128)],
                            prod[(m, t)][base : base + DH, :],
                            start=(idx == 0),
                            stop=(idx == 3),
                            tile_position=(base, 0),
                            skip_group_check=True,
                        )
                nc.vector.tensor_scalar_add(
                    out_sb[:, o, :], pf[(o, 0)][:], bo_t[:, o, :]
                )
                nc.vector.tensor_add(
                    out_sb[:, o, :], out_sb[:, o, :], pf[(o, 64)][:]
                )
                nc.sync.dma_start(outT[ts(o, 128), :], out_sb[:, o, :])

    nc.compile()
    return nc


_NC = None


def _get_nc():
    global _NC
    if _NC is None:
        _NC = _build_graph()
    return _NC


def prep_in_maps(query, key, value, Wq, Wk, Wv, Wo, bo):
    bf = ml_dtypes.bfloat16
    scale = np.float32(1.0 / np.sqrt(DH))

    wqT = np.ascontiguousarray((Wq.astype(np.float32) * scale).T).astype(bf)
    wkT = np.ascontiguousarray(Wk.T).astype(bf)
    # augmented WvT: [257 in-feats (last = ones row), 8 heads x 33 out-cols]
    wvT_a = np.zeros((D + 1, H * (DH + 1)), np.float32)
    wvt = Wv.T.astype(np.float32)  # [in 256, out 256]
    for h in range(H):
        wvT_a[:D, (DH + 1) * h : (DH + 1) * h + DH] = wvt[:, DH * h : DH * (h + 1)]
        wvT_a[D, (DH + 1) * h + DH] = 1.0
    wvT = np.ascontiguousarray(wvT_a).astype(bf)
    # permuted WoT: head h = 4m + 2t + rr lives at partition rows
    # 64*rr .. +32 of free-block 2m+t, matching PV output partitions.
    woP = np.zeros((128, 4, D), np.float32)
    woT = Wo.T.astype(np.float32)  # [hd, out]
    for h in range(H):
        m, r = h // 4, h % 4
        blk, base = 2 * m + r // 2, 64 * (r % 2)
        woP[base : base + DH, blk, :] = woT[DH * h : DH * (h + 1), :]
    woP = np.ascontiguousarray(woP.reshape(128, 4 * D)).astype(bf)
    bo_c = np.ascontiguousarray(bo.astype(np.float32).reshape(D, 1))

    kT_b = [np.ascontiguousarray(key[b].T).astype(bf) for b in range(B)]
    vT_b = [np.ascontiguousarray(value[b].T).astype(bf) for b in range(B)]

    in_maps = []
    for c in range(8):
        b, qb = c // 4, c % 4
        in_maps.append(
            {
                "qT": np.ascontiguousarray(
                    query[b, qb * QB : (qb + 1) * QB, :].T
                ).astype(bf),
                "kT": kT_b[b],
                "vT": vT_b[b],
                "wqT": wqT,
                "wkT": wkT,
                "wvT": wvT,
                "woP": woP,
                "bo": bo_c,
            }
        )
    return in_maps


def kernel(query, key, value, Wq, Wk, Wv, Wo, bo):
    nc = _get_nc()
    in_maps = prep_in_maps(query, key, value, Wq, Wk, Wv, Wo, bo)
    res = run_bass_kernel_spmd(nc, in_maps, core_ids=list(range(8)))

    out = np.empty((B, SEQ, D), np.float32)
    for c in range(8):
        b, qb = c // 4, c % 4
        out[b, qb * QB : (qb + 1) * QB, :] = res.results[c]["outT"].T
    return out


# revision 17
# speedup vs baseline: 1.2105x; 1.0135x over previous
"""Multi-head attention (B=2, S=2048, H=8, Dh=32, D=256) on 8 TRN2 NeuronCores.

Sharding: core c -> (batch b = c//4, query-block qb = c%4 of 512 rows).
Each core computes full attention + output projection for its 512 query rows;
no cross-core communication is needed.  Host does layout prep only
(transposes + bf16 casts); all FLOPs run on device.

Device-side layout (per core):
  - raw activations shipped transposed: qT [256f, 512q], kT/vT [256f, 2048s]
  - q/k projections produced directly transposed (head-dim on partitions,
    head h at partitions 32*(h%4) of free-block h//4) so QK^T runs as
    K=32 row-tiled matmuls; the two heads of a pair are issued
    back-to-back at row positions 32r so the PE runs them concurrently.
  - scores computed TRANSPOSED: scoresT[k, q]; exp is a straight ScalarE
    pass over PSUM (no max subtraction: scores ~ N(0,1)).
  - v projected to natural layout augmented with a ones column per head
    ([128, 16, 8, 33]) so each PV matmul (M=33) also accumulates the
    softmax denominator as its last output row - no separate reduction.
  - normalization: reciprocal of the two denominator rows (partitions 32
    and 96) + K=1 matmul row-broadcast + one DVE multiply per band.
  - final projection: K=32 matmuls per head slice against a host-permuted
    WoT whose row bands match the PV output partition bases.

Schedule (v2): ScalarE is the target critical path (~8.4M exps/core at
1 elem/lane/cycle + ~170cyc/instr overhead -> ~64us).  Its stream carries
ONLY the exp ACTIVATEs (DMA triggers live on sync/vector/gpsimd queues).
Exp regions are [128, 2(heads), 512] PSUM slots (one k-chunk x head-pair,
N=1024); QK production runs 2 slots ahead.  Projections (phase 1-2) and
PV/normalize (phases 2-4) are spread between QK groups so each phase's PE
load roughly matches the ScalarE drain rate, keeping the PE stream dense
(HAM stays released) without starving exp.  The output projection runs in
the tail on freed PSUM slots.  PSUM budget: 3x[128,2,512] rotating slots
(scores + transients + broadcast + final o=0) and 2x[128,512] accumulator
slots (PV quads in flight + final o=1) = exactly 8 banks.
"""

import sys

sys.path.insert(0, "/opt/trn_rl_repo")

import numpy as np
import ml_dtypes

import concourse.bass as bass
import concourse.bacc as bacc
import concourse.mybir as mybir
from concourse.tile import TileContext
from concourse.bass import ts
from concourse.bass_utils import run_bass_kernel_spmd

BF16 = mybir.dt.bfloat16
F32 = mybir.dt.float32
EXP = mybir.ActivationFunctionType.Exp

B, SEQ, D = 2, 2048, 256
H, DH = 8, 32
QB = 512  # query rows per core
NKT = SEQ // 128  # 16 k-chunk tiles (partition tiles of scoresT)


def _build_graph():
    nc = bacc.Bacc("TRN2", target_bir_lowering=False, debug=False)

    qT = nc.declare_dram_parameter("qT", [D, QB], BF16, isOutput=False)
    kT = nc.declare_dram_parameter("kT", [D, SEQ], BF16, isOutput=False)
    vT = nc.declare_dram_parameter("vT", [D, SEQ], BF16, isOutput=False)
    wqT = nc.declare_dram_parameter("wqT", [D, D], BF16, isOutput=False)
    wkT = nc.declare_dram_parameter("wkT", [D, D], BF16, isOutput=False)
    wvT = nc.declare_dram_parameter("wvT", [D + 1, H * (DH + 1)], BF16, isOutput=False)
    woP = nc.declare_dram_parameter("woP", [128, 4 * D], BF16, isOutput=False)
    bo = nc.declare_dram_parameter("bo", [D, 1], F32, isOutput=False)
    outT = nc.declare_dram_parameter("outT", [D, QB], F32, isOutput=True)

    with TileContext(nc) as tc:
        with (
            tc.tile_pool(name="cst", bufs=1) as cst,
            tc.tile_pool(name="sb", bufs=1) as sb,
            tc.tile_pool(name="ps", bufs=2, space="PSUM") as ps,
        ):
            # ---- inputs.  q-path + tail weights ride the scalar HWDGE
            # queue's idle head (all its triggers finish before the first
            # real exp); the k/v bulk rides the sync queue.  kT is split
            # into an early mini-chunk (first score slots) + the rest.
            # The gpsimd SWDGE path is NOT used at all - its slow
            # descriptor-generation dribbles transfers across the whole
            # run and the SBUF write contention slows every ACTIVATE.
            wk_t = cst.tile([128, 2, D], BF16)
            wq_t = cst.tile([128, 2, D], BF16)
            kT_t = cst.tile([128, 2, SEQ], BF16)
            qT_t = cst.tile([128, 2, QB], BF16)
            wv_t = cst.tile([128, 2, H * (DH + 1)], BF16)
            wva_t = cst.tile([1, H * (DH + 1)], BF16)
            vT_t = cst.tile([128, 2, SEQ], BF16)
            wo_t = cst.tile([128, 4, D], BF16)
            bo_t = cst.tile([128, 2, 1], F32)

            for f in range(2):
                nc.scalar.dma_start(wq_t[:, f, :], wqT[ts(f, 128), :])
            for f in range(2):
                nc.scalar.dma_start(qT_t[:, f, :], qT[ts(f, 128), :])
            nc.scalar.dma_start(wo_t[:, :, :], woP.rearrange("p (b d) -> p b d", b=4))
            for o in range(2):
                nc.scalar.dma_start(bo_t[:, o, :], bo[ts(o, 128), :])

            for f in range(2):
                nc.sync.dma_start(wk_t[:, f, :], wkT[ts(f, 128), :])
            for f in range(2):
                nc.sync.dma_start(kT_t[:, f, ts(0, 512)], kT[ts(f, 128), ts(0, 512)])
            for f in range(2):
                nc.sync.dma_start(kT_t[:, f, 512:], kT[ts(f, 128), 512:])
            for f in range(2):
                nc.sync.dma_start(vT_t[:, f, :], vT[ts(f, 128), :])
            for f in range(2):
                nc.sync.dma_start(wv_t[:, f, :], wvT[ts(f, 128), :])
            nc.sync.dma_start(wva_t[:], wvT[D : D + 1, :])

            # warm loads the exp table set early so the ~2.7us
            # ACT_TABLE_LOAD overlaps the DMA/projection phase.
            warm = cst.tile([1, 1], F32)
            nc.vector.memset(warm[:], 0.0)
            nc.scalar.activation(warm[:], warm[:], EXP)

            # ---- SBUF destinations for projections.
            qp = cst.tile([128, 2, QB], BF16)  # q_projT
            kp = cst.tile([128, 2, SEQ], BF16)  # k_projT
            vp = cst.tile([128, NKT, H * (DH + 1)], BF16)  # v_proj + ones cols
            ones_t = cst.tile([128, 64], BF16)
            nc.vector.memset(ones_t[:], 1.0)
            vrow1 = cst.tile([1, SEQ], BF16)
            nc.vector.memset(vrow1[:], 1.0)

            # ---- PE warm-up burst.  The HAM clock gate releases (1.2 ->
            # 2.4 GHz) only after ~3.4us of gapless PE activity; the DMA
            # ramp leaves the PE idle exactly that long, so burn it with
            # back-to-back dummy matmuls (no cross-engine deps between
            # them - same-engine program order keeps the stream dense).
            dmy = cst.tile([128, 512], BF16)
            nc.vector.memset(dmy[:], 0.0)
            pdmy = ps.tile([128, 512], F32, tag="po", bufs=2, name="pdmy")
            for i in range(9):
                nc.tensor.matmul(
                    pdmy[:64, :],
                    ones_t[:, :],
                    dmy[:, :],
                    start=True,
                    stop=True,
                )

            # Transient projection PSUM pieces borrow the rotating "sc"
            # ring (all its occupants are short-lived); the "po" ring is
            # reserved for the long-lived PV accumulators.
            def proj_k(m, s4):
                pk = ps.tile([128, 512], F32, tag="sc", bufs=3, name=f"pk{m}{s4}")
                for f in range(2):
                    nc.tensor.matmul(
                        pk[:],
                        wk_t[:, f, ts(m, 128)],
                        kT_t[:, f, ts(s4, 512)],
                        start=(f == 0),
                        stop=(f == 1),
                    )
                nc.vector.tensor_copy(kp[:, m, ts(s4, 512)], pk[:])

            def proj_q(m):
                pq = ps.tile([128, QB], F32, tag="sc", bufs=3, name=f"pq{m}")
                for f in range(2):
                    nc.tensor.matmul(
                        pq[:],
                        wq_t[:, f, ts(m, 128)],
                        qT_t[:, f, :],
                        start=(f == 0),
                        stop=(f == 1),
                    )
                nc.vector.tensor_copy(qp[:, m, :], pq[:])

            def proj_v(st):
                # third K=1 matmul of the host-side ones row against the
                # augmented Wv row produces the per-head ones columns, so
                # the PSUM->SBUF copy is fully contiguous (strided DVE
                # writes misbehave on HW).
                pv = ps.tile(
                    [128, H * (DH + 1)], F32, tag="sc", bufs=3, name=f"pv{st}"
                )
                for f in range(2):
                    nc.tensor.matmul(
                        pv[:],
                        vT_t[:, f, ts(st, 128)],
                        wv_t[:, f, :],
                        start=(f == 0),
                        stop=False,
                    )
                nc.tensor.matmul(
                    pv[:],
                    vrow1[0:1, ts(st, 128)],
                    wva_t[:],
                    start=False,
                    stop=True,
                )
                nc.vector.tensor_copy(vp[:, st, :], pv[:])

            # attn[(m, t)] = exp(scoresT) for head pair t of quad m:
            # [k-chunk part, ct, r', q]
            attn = {}
            for m in range(2):
                for t in range(2):
                    attn[(m, t)] = sb.tile(
                        [128, NKT, 2, 512], BF16, tag="attn", bufs=4,
                        name=f"attn{m}{t}",
                    )

            def qk_fill(m, t, ct):
                slot = ps.tile(
                    [128, 2, 512], F32, tag="sc", bufs=3, name=f"sc{m}{t}{ct}"
                )
                for rr in range(2):
                    r = 2 * t + rr
                    nc.tensor.matmul(
                        slot[:, rr, :],
                        kp[ts(r, 32), m, ts(ct, 128)],
                        qp[ts(r, 32), m, :],
                        start=True,
                        stop=True,
                        tile_position=(32 * r, 0),
                    )
                return slot

            def qk_act(m, t, ct, slot):
                nc.scalar.activation(attn[(m, t)][:, ct, :, :], slot[:], EXP)

            po = {}  # (m, t) -> PSUM accumulator [128, 512]

            def pv_pair(m, t, ct):
                """PV for both heads of pair (m,t), k-chunk ct.  The two
                matmuls sit at col positions 0/64 so they run concurrently."""
                if ct == 0:
                    po[(m, t)] = ps.tile(
                        [128, 512], F32, tag="po", bufs=2, name=f"po{m}{t}"
                    )
                p = po[(m, t)]
                for rr in range(2):
                    h = 4 * m + 2 * t + rr
                    base = 64 * rr
                    nc.tensor.matmul(
                        p[base : base + DH + 1, :],
                        vp[:, ct, ts(h, DH + 1)],
                        attn[(m, t)][:, ct, rr, :],
                        start=(ct == 0),
                        stop=(ct == NKT - 1),
                        tile_position=(0, base),
                        skip_group_check=True,
                    )

            prod = {}

            def stage_c(m, t, bc=None):
                """normalize: prod = po * (1 / PE-broadcast(denominator rows)).

                The denominator rows sit at partitions 32/96 of each PV
                accumulator; a K=1 matmul against a ones sliver replicates
                each across its head's 32 output partitions (DVE/ACT cannot
                move data across partitions)."""
                p = po[(m, t)]
                dsb = sb.tile([128, 512], BF16, tag="dsb", bufs=2, name=f"dsb{m}{t}")
                if bc is None:
                    bc = ps.tile([128, 512], F32, tag="sc", bufs=3, name=f"bc{m}{t}")
                rsb = sb.tile([128, 512], F32, tag="rsb", bufs=2, name=f"rsb{m}{t}")
                prod[(m, t)] = sb.tile(
                    [128, 512], BF16, tag="prod", bufs=4, name=f"prod{m}{t}"
                )
                for base in (0, 64):
                    nc.vector.tensor_copy(
                        dsb[base + DH : base + DH + 1, :],
                        p[base + DH : base + DH + 1, :],
                    )
                    # M=64 fills bc completely so the full-tile reciprocal
                    # below reads no stale slot bytes.
                    nc.tensor.matmul(
                        bc[base : base + 64, :],
                        ones_t[base + DH : base + DH + 1, :],
                        dsb[base + DH : base + DH + 1, :],
                        start=True,
                        stop=True,
                        tile_position=(base + DH, base),
                        skip_group_check=True,
                    )
                nc.vector.reciprocal_approx_fast(rsb[:], bc[:])
                for base in (0, 64):
                    nc.vector.tensor_mul(
                        prod[(m, t)][base : base + DH, :],
                        p[base : base + DH, :],
                        rsb[base : base + DH, :],
                    )

            # ================= schedule =================
            # startup: q-proj as soon as its (early) DMA lands, a second
            # dummy batch to bridge the PE gap until kT arrives (keeps the
            # HAM released), then the k-proj prefix for the first slots.
            proj_q(0)
            for i in range(6):
                nc.tensor.matmul(
                    pdmy[:64, :],
                    ones_t[:, :],
                    dmy[:, :],
                    start=True,
                    stop=True,
                )
            proj_k(0, 0)

            def PV(m, t, ct):
                return lambda: pv_pair(m, t, ct)

            def SC(m, t):
                return lambda: stage_c(m, t)

            def PK(m, s4):
                return lambda: proj_k(m, s4)

            def PVJ(st):
                return lambda: proj_v(st)

            # per-phase work items: {slot_index: [callables]} + drain list.
            # PE load per slot is kept at or below the ~1us ScalarE drain.
            p1 = {
                0: [PK(0, 1)], 1: [PK(0, 2)], 2: [PK(0, 3)],
                3: [lambda: proj_q(1)],
                4: [PK(1, 0)], 5: [PK(1, 1)], 6: [PK(1, 2)], 7: [PK(1, 3)],
            }
            for i in range(8):
                p1[8 + i] = [PVJ(i)]
            p2 = {i: [PVJ(8 + i)] for i in range(8)}
            for i in range(NKT):
                p2.setdefault(i, []).append(PV(0, 0, i))
            p3 = {0: [SC(0, 0)]}
            for i in range(NKT):
                p3.setdefault(i, []).append(PV(0, 1, i))
            for i in range(3, NKT):
                p3.setdefault(i, []).append(PV(1, 0, i - 3))  # ct 0..12
            p4 = {
                0: [PV(1, 0, 13), SC(0, 1)],
                1: [PV(1, 0, 14)],
                2: [PV(1, 0, 15)],
                3: [SC(1, 0)],
            }
            for i in range(3, NKT):
                p4.setdefault(i, []).append(PV(1, 1, i - 3))  # ct 0..12
            phases = [
                ((0, 0), p1, []),
                ((0, 1), p2, []),
                ((1, 0), p3, []),
                ((1, 1), p4, [PV(1, 1, 13), PV(1, 1, 14), PV(1, 1, 15)]),
            ]

            LOOKAHEAD = 2
            for (m, t), items, drain in phases:
                slots = {}
                for ct in range(LOOKAHEAD):
                    slots[ct] = qk_fill(m, t, ct)
                for ct in range(NKT):
                    if ct + LOOKAHEAD < NKT:
                        slots[ct + LOOKAHEAD] = qk_fill(m, t, ct + LOOKAHEAD)
                    qk_act(m, t, ct, slots.pop(ct))
                    for w in items.get(ct, []):
                        w()
                for w in drain:
                    w()

            # ---- tail.  PSUM tiles in dependency-safe ring order: bc11
            # first (so the last normalize is never gated on the final
            # projection), then the final-projection accumulators.  3/4 of
            # the output projection (prods m0t0/m0t1/m1t0) runs before the
            # last normalize; only (m1,t1)'s K=32 contribution remains
            # serialized after it.
            bc11 = ps.tile([128, 512], F32, tag="sc", bufs=3, name="bc11")
            out_sb = cst.tile([128, 2, QB], F32)
            pf = {}
            pf[(0, 0)] = ps.tile([128, QB], F32, tag="sc", bufs=3, name="pf00")
            pf[(0, 64)] = ps.tile([128, QB], F32, tag="sc", bufs=3, name="pf064")
            pf[(1, 0)] = ps.tile([128, QB], F32, tag="po", bufs=2, name="pf10")
            pf[(1, 64)] = ps.tile([128, QB], F32, tag="sc", bufs=3, name="pf164")

            def pf_mms(idx, m, t):
                for o in range(2):
                    for base in (0, 64):
                        nc.tensor.matmul(
                            pf[(o, base)][:],
                            wo_t[base : base + DH, 2 * m + t, ts(o, 128)],
                            prod[(m, t)][base : base + DH, :],
                            start=(idx == 0),
                            stop=(idx == 3),
                            tile_position=(base, 0),
                            skip_group_check=True,
                        )

            stage_c(1, 1, bc=bc11)
            for idx, (m, t) in enumerate([(0, 0), (0, 1), (1, 0), (1, 1)]):
                pf_mms(idx, m, t)
            for o in range(2):
                nc.vector.tensor_scalar_add(
                    out_sb[:, o, :], pf[(o, 0)][:], bo_t[:, o, :]
                )
                nc.vector.tensor_add(
                    out_sb[:, o, :], out_sb[:, o, :], pf[(o, 64)][:]
                )
                nc.sync.dma_start(outT[ts(o, 128), :], out_sb[:, o, :])

    nc.compile()
    return nc


_NC = None


def _get_nc():
    global _NC
    if _NC is None:
        _NC = _build_graph()
    return _NC


def prep_in_maps(query, key, value, Wq, Wk, Wv, Wo, bo):
    bf = ml_dtypes.bfloat16
    scale = np.float32(1.0 / np.sqrt(DH))

    wqT = np.ascontiguousarray((Wq.astype(np.float32) * scale).T).astype(bf)
    wkT = np.ascontiguousarray(Wk.T).astype(bf)
    # augmented WvT: [257 in-feats (last = ones row), 8 heads x 33 out-cols]
    wvT_a = np.zeros((D + 1, H * (DH + 1)), np.float32)
    wvt = Wv.T.astype(np.float32)  # [in 256, out 256]
    for h in range(H):
        wvT_a[:D, (DH + 1) * h : (DH + 1) * h + DH] = wvt[:, DH * h : DH * (h + 1)]
        wvT_a[D, (DH + 1) * h + DH] = 1.0
    wvT = np.ascontiguousarray(wvT_a).astype(bf)
    # permuted WoT: head h = 4m + 2t + rr lives at partition rows
    # 64*rr .. +32 of free-block 2m+t, matching PV output partitions.
    woP = np.zeros((128, 4, D), np.float32)
    woT = Wo.T.astype(np.float32)  # [hd, out]
    for h in range(H):
        m, r = h // 4, h % 4
        blk, base = 2 * m + r // 2, 64 * (r % 2)
        woP[base : base + DH, blk, :] = woT[DH * h : DH * (h + 1), :]
    woP = np.ascontiguousarray(woP.reshape(128, 4 * D)).astype(bf)
    bo_c = np.ascontiguousarray(bo.astype(np.float32).reshape(D, 1))

    kT_b = [np.ascontiguousarray(key[b].T).astype(bf) for b in range(B)]
    vT_b = [np.ascontiguousarray(value[b].T).astype(bf) for b in range(B)]

    in_maps = []
    for c in range(8):
        b, qb = c // 4, c % 4
        in_maps.append(
            {
                "qT": np.ascontiguousarray(
                    query[b, qb * QB : (qb + 1) * QB, :].T
                ).astype(bf),
                "kT": kT_b[b],
                "vT": vT_b[b],
                "wqT": wqT,
                "wkT": wkT,
                "wvT": wvT,
                "woP": woP,
                "bo": bo_c,
            }
        )
    return in_maps


def kernel(query, key, value, Wq, Wk, Wv, Wo, bo):
    nc = _get_nc()
    in_maps = prep_in_maps(query, key, value, Wq, Wk, Wv, Wo, bo)
    res = run_bass_kernel_spmd(nc, in_maps, core_ids=list(range(8)))

    out = np.empty((B, SEQ, D), np.float32)
    for c in range(8):
        b, qb = c // 4, c % 4
        out[b, qb * QB : (qb + 1) * QB, :] = res.results[c]["outT"].T
    return out
